# revision 2
# baseline (speedup 1.0000x reference)
"""Chamfer loss (nn_ChamferLoss_45157286150461) Trainium2 Bass kernel, v2.

Math (matches the reference):
    P[b,i,j] = ||gts[b,i]||^2 + ||preds[b,j]||^2 - 2 gts[b,i].preds[b,j]
    out = mean_j min_i P  +  mean_i min_j P       (means over all b,j / b,i)

Sharding: data-parallel over batch. 8 cores x 2 batches each. Each core
returns one f32 partial = sum(min_i P) + sum(min_j P) over its two
batches; the host sums the 8 partials and divides by B*N.

v2 changes vs v1:
  - ALL input prep on host: U/V augmented fp16 matrices (hi/lo split,
    norms) assembled in numpy; device loads one [52,4096] f16 tile.
  - PSUM groups of [128,2048] f32 (4 banks), 2 bufs = all 8 banks;
    ScalarE drains each group in one fd=2048 activation (measured
    1343ns vs 4x396ns at fd=512).
  - dl epilogue transposes via DMA (frees PSUM banks), final partition
    sum via a tiny f32 ones-matmul into a corner of a rotating ps tile.

Per batch: P = U^T V computed 128 rows x 2048 cols at a time (fp16
augmented matmul, K=13); ScalarE drains PSUM -> fp16 SBUF; VectorE
does dr (7 chunked fd=512 tensor_tensor mins + one fd=512 reduce) and
dl (tensor_tensor min-accumulate into M[128,4096]).
"""

import os
import sys
from contextlib import ExitStack

for _p in ("/opt/trn_rl_repo", "/root/.axon_site/_ro/trn_rl_repo"):
    if os.path.isdir(_p) and _p not in sys.path:
        sys.path.insert(0, _p)

import numpy as np

import concourse.bass as bass  # noqa: F401
import concourse.tile as tile
from concourse import bacc, mybir
from concourse.bass_utils import run_bass_kernel_spmd

f32 = mybir.dt.float32
f16 = mybir.dt.float16
AX = mybir.AxisListType
OP = mybir.AluOpType
ACTF = mybir.ActivationFunctionType

N_CORES = 8
B = 16
N = 4096
D = 3
BPC = B // N_CORES  # batches per core
P = 128             # i-tile (PSUM partition dim)
JW = 512            # j-tile per matmul
JG = int(os.environ.get("CH2_JG", "1024"))  # j-group per PSUM tile
NIT = N // P        # 32
NJG = N // JG
KC = 13             # augmented contraction rows


def build_program(do_compile=True, loop_reps=None, unroll_reps=1):
    dlw = int(os.environ.get("CH2_DLW", "4096"))  # dl TT op width
    tbufs = int(os.environ.get("CH2_TBUFS", "4"))
    dl_first = os.environ.get("CH2_DLFIRST", "1") == "1"

    nc = bacc.Bacc("TRN2", target_bir_lowering=False, debug=False)

    uv_d = nc.dram_tensor("uv", [4 * KC, N], f16, kind="ExternalInput")
    out_d = nc.dram_tensor("out", [1, 1], f32, kind="ExternalOutput")

    with ExitStack() as ctx:
        tc = ctx.enter_context(tile.TileContext(nc))
        consts = ctx.enter_context(tc.tile_pool(name="consts", bufs=1))
        mpool = ctx.enter_context(tc.tile_pool(name="mmin", bufs=2))
        tpool = ctx.enter_context(tc.tile_pool(name="tconv", bufs=tbufs))
        accp = ctx.enter_context(tc.tile_pool(name="acc", bufs=2))
        trp = ctx.enter_context(tc.tile_pool(name="trsb", bufs=8))
        resp = ctx.enter_context(tc.tile_pool(name="res", bufs=1))
        psbufs = int(os.environ.get("CH2_PSBUFS", str(8 * 512 // JG)))
        psA = ctx.enter_context(tc.tile_pool(name="psA", bufs=psbufs, space="PSUM"))

        qs0 = [nc.sync, nc.scalar]
        uvt = []
        for i in range(4):
            t = consts.tile([KC, N], f16, name=f"uv{i}", tag=f"uv{i}")
            qs0[i % 2].dma_start(t[:], uv_d[i * KC : (i + 1) * KC, :])
            uvt.append(t)
        ones_col = consts.tile([P, 1], f32)
        nc.vector.memset(ones_col[:], 1.0)
        res = resp.tile([1, BPC], f32)

        if loop_reps is not None:
            ctx.enter_context(tc.For_i(0, loop_reps, 1))

        qs = [nc.sync, nc.scalar]

        for b in [bb for _ in range(unroll_reps) for bb in range(BPC)]:
            U, V = uvt[b], uvt[2 + b]
            M = mpool.tile([P, N], f16, tag="M")
            DR = accp.tile([P, NIT], f32, tag="DR")
            for it in range(NIT):
                lhsT = U[:, it * P : (it + 1) * P]
                if it == 0:
                    T = M
                else:
                    T = tpool.tile([P, N], f16, tag="T")
                for jg in range(NJG):
                    ps = psA.tile([P, JG], f32, tag="ps")
                    for h in range(JG // JW):
                        j0 = jg * JG + h * JW
                        nc.tensor.matmul(
                            ps[:, h * JW : (h + 1) * JW],
                            lhsT,
                            V[:, j0 : j0 + JW],
                            start=True,
                            stop=True,
                        )
                    nc.scalar.activation(
                        T[:, jg * JG : (jg + 1) * JG], ps[:], ACTF.Copy
                    )

                def emit_dr():
                    drmode = os.environ.get("CH2_DR", "chain")
                    ch = [T[:, c * JW : (c + 1) * JW] for c in range(N // JW)]
                    if drmode == "2chain":
                        # two interleaved fold chains: consecutive DVE ops
                        # are independent (serial in-place chains are
                        # latency-bound: 442ns vs 313ns per op)
                        RA = accp.tile([P, JW], f16, tag="RA")
                        RB = accp.tile([P, JW], f16, tag="RB")
                        nc.vector.tensor_tensor(RA[:], ch[0], ch[2], op=OP.min)
                        nc.vector.tensor_tensor(RB[:], ch[1], ch[3], op=OP.min)
                        nc.vector.tensor_tensor(RA[:], RA[:], ch[4], op=OP.min)
                        nc.vector.tensor_tensor(RB[:], RB[:], ch[5], op=OP.min)
                        nc.vector.tensor_tensor(RA[:], RA[:], ch[6], op=OP.min)
                        nc.vector.tensor_tensor(RB[:], RB[:], ch[7], op=OP.min)
                        nc.vector.tensor_tensor(RA[:], RA[:], RB[:], op=OP.min)
                        R = RA
                    else:
                        R = accp.tile([P, JW], f16, tag="R")
                        nc.vector.tensor_tensor(R[:], ch[0], ch[1], op=OP.min)
                        for c in range(2, N // JW):
                            nc.vector.tensor_tensor(R[:], R[:], ch[c], op=OP.min)
                    nc.vector.tensor_reduce(
                        DR[:, it : it + 1], R[:], axis=AX.X, op=OP.min
                    )

                def emit_dl():
                    if it == 0:
                        return
                    for c0 in range(0, N, dlw):
                        nc.vector.tensor_tensor(
                            M[:, c0 : c0 + dlw],
                            T[:, c0 : c0 + dlw],
                            M[:, c0 : c0 + dlw],
                            op=OP.min,
                        )

                if dl_first:
                    emit_dl()
                    emit_dr()
                else:
                    emit_dr()
                    emit_dl()

            # ---- dl: min over partitions via DMA transpose + reduce ----
            DL = accp.tile([P, NIT], f16, tag="DL")
            for k in range(NIT):
                tp = trp.tile([P, P], f16, tag="TP")
                qs[k % 2].dma_start(tp[:], M[:, k * P : (k + 1) * P], transpose=True)
                nc.vector.tensor_reduce(
                    DL[:, k : k + 1], tp[:], axis=AX.X, op=OP.min
                )

            # ---- sums ----
            sm = accp.tile([P, 2], f32, tag="sm")
            nc.vector.tensor_reduce(sm[:, 0:1], DR[:], axis=AX.X, op=OP.add)
            nc.vector.tensor_reduce(sm[:, 1:2], DL[:], axis=AX.X, op=OP.add)
            sv = accp.tile([P, 1], f32, tag="sv")
            nc.vector.tensor_reduce(sv[:], sm[:], axis=AX.X, op=OP.add)
            ps = psA.tile([P, JG], f32, tag="ps")
            nc.tensor.matmul(
                ps[0:1, 0:1], sv[:], ones_col[:], start=True, stop=True
            )
            nc.scalar.activation(res[:, b : b + 1], ps[0:1, 0:1], ACTF.Copy)

        outsb = resp.tile([1, 1], f32)
        nc.vector.tensor_reduce(outsb[:], res[:], axis=AX.X, op=OP.add)
        nc.sync.dma_start(out_d[:], outsb[:])

    if do_compile:
        nc.compile()
    return nc


def _split16(a):
    h = a.astype(np.float16)
    l = (a - h.astype(np.float32)).astype(np.float16)
    return h, l


def make_in_maps(preds, gts):
    in_maps = []
    ones = np.ones((1, N), np.float16)
    for c in range(N_CORES):
        rows = []
        for b in range(BPC):
            g = np.asarray(gts[c * BPC + b], np.float32)      # x = gts
            xs = (-2.0 * g.T)                                  # [3, N]
            xs_h, xs_l = _split16(xs)
            sx = (g.astype(np.float64) ** 2).sum(-1).astype(np.float32)[None, :]
            sx_h, sx_l = _split16(sx)
            rows.append(np.concatenate(
                [xs_h, xs_h, xs_l, sx_h, sx_l, ones, ones], axis=0))
        for b in range(BPC):
            y = np.asarray(preds[c * BPC + b], np.float32).T   # [3, N]
            y_h, y_l = _split16(y)
            sy = (np.asarray(preds[c * BPC + b], np.float64) ** 2).sum(
                -1).astype(np.float32)[None, :]
            sy_h, sy_l = _split16(sy)
            rows.append(np.concatenate(
                [y_h, y_l, y_h, ones, ones, sy_h, sy_l], axis=0))
        uv = np.ascontiguousarray(np.concatenate(rows, axis=0))
        assert uv.shape == (4 * KC, N)
        in_maps.append({"uv": uv})
    return in_maps


_prog = None
last_run_info = {}


def kernel(preds, gts):
    global _prog
    preds = np.ascontiguousarray(np.asarray(preds, dtype=np.float32))
    gts = np.ascontiguousarray(np.asarray(gts, dtype=np.float32))
    assert preds.shape == (B, N, D) and gts.shape == (B, N, D)
    if _prog is None:
        _prog = build_program()
    in_maps = make_in_maps(preds, gts)
    trace = bool(int(os.environ.get("CHAMFER_TRACE", "0")))
    r = run_bass_kernel_spmd(_prog, in_maps, list(range(N_CORES)), trace=trace)
    last_run_info["exec_time_ns"] = r.exec_time_ns
    last_run_info["results"] = r
    total = sum(float(m["out"][0, 0]) for m in r.results)
    return np.asarray(total / float(B * N), dtype=np.float32)


# revision 4
# speedup vs baseline: 1.0813x; 1.0813x over previous
"""Chamfer loss (nn_ChamferLoss_45157286150461) Trainium2 Bass kernel, v2.

Math (matches the reference):
    P[b,i,j] = ||gts[b,i]||^2 + ||preds[b,j]||^2 - 2 gts[b,i].preds[b,j]
    out = mean_j min_i P  +  mean_i min_j P       (means over all b,j / b,i)

Sharding: data-parallel over batch. 8 cores x 2 batches each. Each core
returns one f32 partial = sum(min_i P) + sum(min_j P) over its two
batches; the host sums the 8 partials and divides by B*N.

v2 changes vs v1:
  - ALL input prep on host: U/V augmented fp16 matrices (hi/lo split,
    norms) assembled in numpy; device loads one [52,4096] f16 tile.
  - PSUM groups of [128,2048] f32 (4 banks), 2 bufs = all 8 banks;
    ScalarE drains each group in one fd=2048 activation (measured
    1343ns vs 4x396ns at fd=512).
  - dl epilogue transposes via DMA (frees PSUM banks), final partition
    sum via a tiny f32 ones-matmul into a corner of a rotating ps tile.

Per batch: P = U^T V computed 128 rows x 2048 cols at a time (fp16
augmented matmul, K=13); ScalarE drains PSUM -> fp16 SBUF; VectorE
does dr (7 chunked fd=512 tensor_tensor mins + one fd=512 reduce) and
dl (tensor_tensor min-accumulate into M[128,4096]).
"""

import os
import sys
from contextlib import ExitStack

for _p in ("/opt/trn_rl_repo", "/root/.axon_site/_ro/trn_rl_repo"):
    if os.path.isdir(_p) and _p not in sys.path:
        sys.path.insert(0, _p)

import numpy as np

import concourse.bass as bass  # noqa: F401
import concourse.tile as tile
from concourse import bacc, mybir
from concourse.bass_utils import run_bass_kernel_spmd

f32 = mybir.dt.float32
f16 = mybir.dt.float16
AX = mybir.AxisListType
OP = mybir.AluOpType
ACTF = mybir.ActivationFunctionType

N_CORES = 8
B = 16
N = 4096
D = 3
BPC = B // N_CORES  # batches per core
P = 128             # i-tile (PSUM partition dim)
JW = 512            # j-tile per matmul
JG = int(os.environ.get("CH2_JG", "1024"))  # j-group per PSUM tile
NIT = N // P        # 32
NJG = N // JG
KC = 13             # augmented contraction rows


def build_program(do_compile=True, loop_reps=None, unroll_reps=1):
    dlw = int(os.environ.get("CH2_DLW", "4096"))  # dl TT op width
    tbufs = int(os.environ.get("CH2_TBUFS", "4"))
    dl_first = os.environ.get("CH2_DLFIRST", "1") == "1"

    nc = bacc.Bacc("TRN2", target_bir_lowering=False, debug=False)

    uv_d = nc.dram_tensor("uv", [4 * KC, N], f16, kind="ExternalInput")
    out_d = nc.dram_tensor("out", [1, 1], f32, kind="ExternalOutput")

    with ExitStack() as ctx:
        tc = ctx.enter_context(tile.TileContext(nc))
        consts = ctx.enter_context(tc.tile_pool(name="consts", bufs=1))
        mpool = ctx.enter_context(tc.tile_pool(name="mmin", bufs=2))
        tpool = ctx.enter_context(tc.tile_pool(name="tconv", bufs=tbufs))
        accp = ctx.enter_context(tc.tile_pool(name="acc", bufs=2))
        trp = ctx.enter_context(tc.tile_pool(name="trsb", bufs=8))
        resp = ctx.enter_context(tc.tile_pool(name="res", bufs=1))
        psbufs = int(os.environ.get("CH2_PSBUFS", str(8 * 512 // JG)))
        psA = ctx.enter_context(tc.tile_pool(name="psA", bufs=psbufs, space="PSUM"))

        qs0 = [nc.sync, nc.scalar]
        uvt = []
        for i in range(4):
            t = consts.tile([KC, N], f16, name=f"uv{i}", tag=f"uv{i}")
            qs0[i % 2].dma_start(t[:], uv_d[i * KC : (i + 1) * KC, :])
            uvt.append(t)
        ones_col = consts.tile([P, 1], f32)
        nc.vector.memset(ones_col[:], 1.0)
        res = resp.tile([1, BPC], f32)

        if loop_reps is not None:
            ctx.enter_context(tc.For_i(0, loop_reps, 1))

        qs = [nc.sync, nc.scalar]
        interleave = os.environ.get("CH2_INTERLEAVE", "1") == "1"

        def emit_tile(b, it, M, DR):
            U, V = uvt[b], uvt[2 + b]
            lhsT = U[:, it * P : (it + 1) * P]
            if it == 0:
                T = M
            else:
                T = tpool.tile([P, N], f16, tag="T")
            for jg in range(NJG):
                ps = psA.tile([P, JG], f32, tag="ps")
                for h in range(JG // JW):
                    j0 = jg * JG + h * JW
                    nc.tensor.matmul(
                        ps[:, h * JW : (h + 1) * JW],
                        lhsT,
                        V[:, j0 : j0 + JW],
                        start=True,
                        stop=True,
                    )
                nc.scalar.activation(
                    T[:, jg * JG : (jg + 1) * JG], ps[:], ACTF.Copy
                )

            def emit_dr():
                drmode = os.environ.get("CH2_DR", "chain")
                ch = [T[:, c * JW : (c + 1) * JW] for c in range(N // JW)]
                if drmode == "2chain":
                    # two interleaved fold chains: consecutive DVE ops
                    # are independent (serial in-place chains are
                    # latency-bound: 442ns vs 313ns per op)
                    RA = accp.tile([P, JW], f16, tag="RA")
                    RB = accp.tile([P, JW], f16, tag="RB")
                    nc.vector.tensor_tensor(RA[:], ch[0], ch[2], op=OP.min)
                    nc.vector.tensor_tensor(RB[:], ch[1], ch[3], op=OP.min)
                    nc.vector.tensor_tensor(RA[:], RA[:], ch[4], op=OP.min)
                    nc.vector.tensor_tensor(RB[:], RB[:], ch[5], op=OP.min)
                    nc.vector.tensor_tensor(RA[:], RA[:], ch[6], op=OP.min)
                    nc.vector.tensor_tensor(RB[:], RB[:], ch[7], op=OP.min)
                    nc.vector.tensor_tensor(RA[:], RA[:], RB[:], op=OP.min)
                    R = RA
                else:
                    R = accp.tile([P, JW], f16, tag="R")
                    nc.vector.tensor_tensor(R[:], ch[0], ch[1], op=OP.min)
                    for c in range(2, N // JW):
                        nc.vector.tensor_tensor(R[:], R[:], ch[c], op=OP.min)
                nc.vector.tensor_reduce(
                    DR[:, it : it + 1], R[:], axis=AX.X, op=OP.min
                )

            def emit_dl():
                if it == 0:
                    return
                for c0 in range(0, N, dlw):
                    nc.vector.tensor_tensor(
                        M[:, c0 : c0 + dlw],
                        T[:, c0 : c0 + dlw],
                        M[:, c0 : c0 + dlw],
                        op=OP.min,
                    )

            if dl_first:
                emit_dl()
                emit_dr()
            else:
                emit_dr()
                emit_dl()

        def emit_epilogue(b, M, DR):
            # ---- dl: min over partitions via DMA transpose + reduce ----
            DL = accp.tile([P, NIT], f16, tag="DL")
            for k in range(NIT):
                tp = trp.tile([P, P], f16, tag="TP")
                qs[k % 2].dma_start(tp[:], M[:, k * P : (k + 1) * P], transpose=True)
                nc.vector.tensor_reduce(
                    DL[:, k : k + 1], tp[:], axis=AX.X, op=OP.min
                )
            # ---- sums ----
            sm = accp.tile([P, 2], f32, tag="sm")
            nc.vector.tensor_reduce(sm[:, 0:1], DR[:], axis=AX.X, op=OP.add)
            nc.vector.tensor_reduce(sm[:, 1:2], DL[:], axis=AX.X, op=OP.add)
            sv = accp.tile([P, 1], f32, tag="sv")
            nc.vector.tensor_reduce(sv[:], sm[:], axis=AX.X, op=OP.add)
            ps = psA.tile([P, JG], f32, tag="ps")
            nc.tensor.matmul(
                ps[0:1, 0:1], sv[:], ones_col[:], start=True, stop=True
            )
            nc.scalar.activation(res[:, b : b + 1], ps[0:1, 0:1], ACTF.Copy)

        for _ in range(unroll_reps):
            if interleave:
                Ms = [mpool.tile([P, N], f16, name=f"M{b}", tag="M")
                      for b in range(BPC)]
                DRs = [accp.tile([P, NIT], f32, name=f"DR{b}", tag="DR")
                       for b in range(BPC)]
                for it in range(NIT):
                    for b in range(BPC):
                        emit_tile(b, it, Ms[b], DRs[b])
                for b in range(BPC):
                    emit_epilogue(b, Ms[b], DRs[b])
            else:
                for b in range(BPC):
                    M = mpool.tile([P, N], f16, tag="M")
                    DR = accp.tile([P, NIT], f32, tag="DR")
                    for it in range(NIT):
                        emit_tile(b, it, M, DR)
                    emit_epilogue(b, M, DR)

        outsb = resp.tile([1, 1], f32)
        nc.vector.tensor_reduce(outsb[:], res[:], axis=AX.X, op=OP.add)
        nc.sync.dma_start(out_d[:], outsb[:])

    if do_compile:
        nc.compile()
    return nc


def _split16(a):
    h = a.astype(np.float16)
    l = (a - h.astype(np.float32)).astype(np.float16)
    return h, l


def make_in_maps(preds, gts):
    in_maps = []
    ones = np.ones((1, N), np.float16)
    for c in range(N_CORES):
        rows = []
        for b in range(BPC):
            g = np.asarray(gts[c * BPC + b], np.float32)      # x = gts
            xs = (-2.0 * g.T)                                  # [3, N]
            xs_h, xs_l = _split16(xs)
            sx = (g.astype(np.float64) ** 2).sum(-1).astype(np.float32)[None, :]
            sx_h, sx_l = _split16(sx)
            rows.append(np.concatenate(
                [xs_h, xs_h, xs_l, sx_h, sx_l, ones, ones], axis=0))
        for b in range(BPC):
            y = np.asarray(preds[c * BPC + b], np.float32).T   # [3, N]
            y_h, y_l = _split16(y)
            sy = (np.asarray(preds[c * BPC + b], np.float64) ** 2).sum(
                -1).astype(np.float32)[None, :]
            sy_h, sy_l = _split16(sy)
            rows.append(np.concatenate(
                [y_h, y_l, y_h, ones, ones, sy_h, sy_l], axis=0))
        uv = np.ascontiguousarray(np.concatenate(rows, axis=0))
        assert uv.shape == (4 * KC, N)
        in_maps.append({"uv": uv})
    return in_maps


_prog = None
last_run_info = {}


def kernel(preds, gts):
    global _prog
    preds = np.ascontiguousarray(np.asarray(preds, dtype=np.float32))
    gts = np.ascontiguousarray(np.asarray(gts, dtype=np.float32))
    assert preds.shape == (B, N, D) and gts.shape == (B, N, D)
    if _prog is None:
        _prog = build_program()
    in_maps = make_in_maps(preds, gts)
    trace = bool(int(os.environ.get("CHAMFER_TRACE", "0")))
    r = run_bass_kernel_spmd(_prog, in_maps, list(range(N_CORES)), trace=trace)
    last_run_info["exec_time_ns"] = r.exec_time_ns
    last_run_info["results"] = r
    total = sum(float(m["out"][0, 0]) for m in r.results)
    return np.asarray(total / float(B * N), dtype=np.float32)
